# revision 39
# baseline (speedup 1.0000x reference)
"""Distributed Bass kernel for nn_AttentionLayer (B=2, S=2048, H=1024, NH=16).

Sharding: 8 cores = 2 batch groups x 4 ranks. Core c handles batch c//4 and
heads [4r:4r+4] (r = c%4). QKV projections are column-sharded; attention runs
per-head with a transposed dataflow (scores^T so softmax's reduction axis sits
on PSUM partitions and feeds the ctx matmul directly). The rank exchange moves
ctx (1 MB via AllToAll) instead of out-proj partials; each rank then computes
the full 16-head output projection + residual + LayerNorm on its 512-token
shard.

v2: all heavy matmuls except QK^T run as fp8e4m3 DoubleRow (contract 256 per
pass -> 2x PE): QKV projections pair k-tiles, ctx pairs key chunks, out-proj
pairs head pairs. x / Wq/Wk/Wv/Wo are pre-transposed + pre-quantized to fp8 on
the host, which also kills the on-device transposes and staging copies.
Softmax stays unnormalized-exp + deferred renorm (ones-column in V gives exp
sums as PSUM row 64); exp emits fp8e5m2 (ACT exact-exp for the even head,
uint8 Schraudolph bitcast on DVE for the odd head) feeding the DoubleRow ctx
matmul directly. QK^T stays bf16 (HD=64 contract gains nothing from fp8).
Attention is exp-bound (ACT+DVE saturated); everything else is scheduled
around that: the renorm's Z-broadcast goes through a tiny selector matmul
into the pinned ctx tiles' unused partitions (no Pool dependency — Pool
stalls behind in-flight collectives), SBUF-only work (pair-0 staging, xpb,
recv1 sums) rides Pool, and recv0's collective-blocked DMAs carry a
scheduler wait floor + run on the ACT queue so they can't wedge the SP
stream mid-attention. The second AllToAll fires immediately after the last
quarter's renorm; the even-pair out-proj phase + residual prep fill its
window, so only recv1+odd-phase+LN sit after it.
"""

import sys
from contextlib import ExitStack

sys.path.insert(0, "/opt/trn_rl_repo")

import numpy as np
from concourse import bacc, bass, bass_utils, mybir, tile

AF = mybir.ActivationFunctionType
ALU = mybir.AluOpType
DR = mybir.MatmulPerfMode.DoubleRow
F32 = mybir.dt.float32
BF16 = mybir.dt.bfloat16
E4 = mybir.dt.float8e4
E5 = mybir.dt.float8e5
U8 = mybir.dt.uint8

B, S, H, NH, HD = 2, 2048, 1024, 16, 64
N_CORES = 8
RANKS = 4  # ranks per batch group
GROUPS = [[0, 1, 2, 3], [4, 5, 6, 7]]
HPC = NH // RANKS  # heads per core = 4
DLOC = HPC * HD  # local head dims = 256
SSH = S // RANKS  # token shard = 512
LN_EPS = 1e-5
P = 128
KO = H // P  # 8 k-tiles over hidden dim
TI = S // P  # 16 token tiles
QHN = 512  # attention query-quarter width
NKC = S // P  # 16 key chunks
SCL = 0.125  # 1/sqrt(HD)
SCH_A5 = (4.0 / np.log(2.0)) * SCL  # Schraudolph slope for e5m2 (scale folded)
SCH_B5 = 60.0 - 0.22  # e5m2 offset, C tuned for RN convert
RECV0_WAIT_MS = 0.130  # scheduler floor for recv0 (keeps its collective-
                       # blocked DMAs out of the attention SP stream)


def build(no_collective=False):
    nc = bacc.Bacc("TRN2", target_bir_lowering=False, debug=False, num_devices=N_CORES)

    x8_d = nc.dram_tensor("x8", [P, KO, S], E4, kind="ExternalInput")
    xres = nc.dram_tensor("xres", [SSH, H], F32, kind="ExternalInput")
    mask_d = nc.dram_tensor("mask", [N_CORES], F32, kind="ExternalInput")
    wq_d = nc.dram_tensor("wq", [P, KO, DLOC], E4, kind="ExternalInput")
    wk_d = nc.dram_tensor("wk", [P, KO, DLOC], E4, kind="ExternalInput")
    wv_d = nc.dram_tensor("wv", [P, KO, DLOC], E4, kind="ExternalInput")
    wo_d = nc.dram_tensor("wo", [P, NH // 2, H], E4, kind="ExternalInput")
    bq_d = nc.dram_tensor("bq", [DLOC], F32, kind="ExternalInput")
    bk_d = nc.dram_tensor("bk", [DLOC], F32, kind="ExternalInput")
    bv_d = nc.dram_tensor("bv", [DLOC], F32, kind="ExternalInput")
    bo_d = nc.dram_tensor("bo", [H], F32, kind="ExternalInput")
    gamma_d = nc.dram_tensor("gamma", [H], F32, kind="ExternalInput")
    beta_d = nc.dram_tensor("beta", [H], F32, kind="ExternalInput")
    pat2_d = nc.dram_tensor("pat2", [2, P], BF16, kind="ExternalInput")
    out_d = nc.dram_tensor("out", [SSH, H], F32, kind="ExternalOutput")

    with tile.TileContext(nc) as tc, ExitStack() as ctx:
        _build_body(
            nc, tc, ctx,
            x8_d, xres, mask_d, wq_d, wk_d, wv_d, wo_d, bq_d, bk_d, bv_d, bo_d,
            gamma_d, beta_d, pat2_d, out_d, no_collective=no_collective,
        )
    nc.compile()
    return nc


def _build_body(
    nc, tc, ctx, x8_d, xres, mask_d, wq_d, wk_d, wv_d, wo_d, bq_d, bk_d, bv_d,
    bo_d, gamma_d, beta_d, pat2_d, out_d, no_collective=False,
):
    const = ctx.enter_context(tc.tile_pool(name="const", bufs=1))
    stg = ctx.enter_context(tc.tile_pool(name="stg", bufs=2))
    expp = ctx.enter_context(tc.tile_pool(name="expp", bufs=6))
    small = ctx.enter_context(tc.tile_pool(name="small", bufs=2))
    epi = ctx.enter_context(tc.tile_pool(name="epi", bufs=2))
    dram = ctx.enter_context(tc.tile_pool(name="dram", bufs=1, space="DRAM"))
    psS = ctx.enter_context(tc.tile_pool(name="psS", bufs=4, space="PSUM"))
    psC = ctx.enter_context(tc.tile_pool(name="psC", bufs=4, space="PSUM"))
    a2ap = ctx.enter_context(tc.tile_pool(name="a2ap", bufs=3))

    a2a_in0 = dram.tile([N_CORES, P, SSH], E4, tag="a2a_in0")
    a2a_out0 = dram.tile([N_CORES, P, SSH], E4, tag="a2a_out0")
    a2a_in1 = dram.tile([N_CORES, P, SSH], E4, tag="a2a_in1")
    a2a_out1 = dram.tile([N_CORES, P, SSH], E4, tag="a2a_out1")

    # ---- front DMAs, spread across engine queues ----
    w8q = const.tile([P, KO, DLOC], E4, tag="w8q")
    w8k = const.tile([P, KO, DLOC], E4, tag="w8k")
    w8v = const.tile([P, KO, DLOC], E4, tag="w8v")
    nc.sync.dma_start(w8q[:], wq_d[:])
    nc.sync.dma_start(w8k[:], wk_d[:])
    nc.sync.dma_start(w8v[:], wv_d[:])

    x8 = const.tile([P, KO, S], E4, tag="x8")
    for c2 in range(0, KO, 2):
        eng = nc.sync if c2 < KO // 2 else nc.scalar
        eng.dma_start(x8[:, c2 : c2 + 2], x8_d[:, c2 : c2 + 2])

    wo8 = const.tile([P, NH // 2, H], E4, tag="wo8")
    for c4 in range(0, NH // 2, 4):
        nc.gpsimd.dma_start(wo8[:, c4 : c4 + 4], wo_d[:, c4 : c4 + 4])

    # per-partition biases for Q/K projections: [DLOC] -> [P, 2]
    bq_sb = const.tile([P, DLOC // P], F32)
    nc.sync.dma_start(bq_sb[:], bq_d[:].rearrange("(o p) -> p o", p=P))
    bk_sb = const.tile([P, DLOC // P], F32)
    nc.sync.dma_start(bk_sb[:], bk_d[:].rearrange("(o p) -> p o", p=P))

    # free-axis vectors, replicated across partitions via gpsimd
    def bcast_vec(dram_t, n):
        row = stg.tile([1, n], F32, tag="wstg")
        nc.sync.dma_start(row[:], dram_t[:].rearrange("(o n) -> o n", o=1))
        bc = const.tile([P, n], F32, tag=f"bc_{dram_t.name}")
        nc.gpsimd.partition_broadcast(bc[:], row[:])
        return bc

    bv_bc = bcast_vec(bv_d, DLOC)
    bo_bc = bcast_vec(bo_d, H)
    gamma_bc = bcast_vec(gamma_d, H)
    beta_bc = bcast_vec(beta_d, H)
    eps_sb = const.tile([P, 1], F32)
    nc.vector.memset(eps_sb[:], LN_EPS)

    maskb = const.tile([P, N_CORES], F32)
    mrow = stg.tile([1, N_CORES], F32, tag="wstg")
    nc.sync.dma_start(mrow[:], mask_d[:].rearrange("(o n) -> o n", o=1))
    nc.gpsimd.partition_broadcast(maskb[:], mrow[:])

    # renorm broadcast selector: zb[p] = Z0 for p<64, Z1 for p>=64 (fp32 PE)
    pat2 = const.tile([2, P], BF16, tag="pat2")
    nc.sync.dma_start(pat2[:], pat2_d[:])

    # ---- V in fp8e4m3 with a ones column at HD (exp-sums trick) ----
    v8 = const.tile([P, TI, HPC, P], E4)
    nc.gpsimd.memset(v8[:, :, :, HD], 1.0)
    nc.gpsimd.memset(v8[:, :, :, HD + 1 :], 0.0)

    # ---- projections (fp8 DoubleRow: contract 256 per matmul) ----
    QT = const.tile([P, DLOC // P, S], BF16)
    KT = const.tile([P, DLOC // P, S], BF16)

    def qk_proj(dst, w_sb, b_sb, pr, q4, alt):
        col = q4 * QHN
        ps = psS.tile([P, QHN], F32, tag="ps", name=f"qk{pr}_{q4}_{id(dst) % 97}")
        for kop in range(KO // 2):
            nc.tensor.matmul(
                ps[:],
                w_sb[:, 2 * kop : 2 * kop + 2, pr * P : (pr + 1) * P],
                x8[:, 2 * kop : 2 * kop + 2, col : col + QHN],
                start=(kop == 0),
                stop=(kop == KO // 2 - 1),
                perf_mode=DR,
            )
        # bias + bf16 convert, alternating ACT/DVE so neither gates the front
        if alt % 2 == 0:
            nc.scalar.activation(
                dst[:, pr, col : col + QHN], ps[:], AF.Identity,
                bias=b_sb[:, pr : pr + 1], scale=1.0,
            )
        else:
            nc.vector.tensor_scalar(
                out=dst[:, pr, col : col + QHN], in0=ps[:],
                scalar1=b_sb[:, pr : pr + 1], scalar2=None, op0=ALU.add,
            )

    def v_build(ti):
        ps = psS.tile([P, DLOC], F32, tag="ps", name=f"vb{ti}")
        for kop in range(KO // 2):
            nc.tensor.matmul(
                ps[:, :DLOC],
                x8[:, 2 * kop : 2 * kop + 2, ti * P : (ti + 1) * P],
                w8v[:, 2 * kop : 2 * kop + 2, :],
                start=(kop == 0),
                stop=(kop == KO // 2 - 1),
                perf_mode=DR,
            )
        nc.vector.tensor_tensor(
            v8[:, ti, :, :HD],
            ps[:, :DLOC].rearrange("p (h d) -> p h d", h=HPC),
            bv_bc[:].rearrange("p (h d) -> p h d", h=HPC),
            ALU.add,
        )

    for q4 in range(4):
        qk_proj(QT, w8q, bq_sb, 0, q4, 2 * q4)
        qk_proj(KT, w8k, bk_sb, 0, q4, 2 * q4 + 1)
    for ti in range(TI):
        v_build(ti)
    for q4 in range(4):
        qk_proj(QT, w8q, bq_sb, 1, q4, 2 * q4)
        qk_proj(KT, w8k, bk_sb, 1, q4, 2 * q4 + 1)

    # ---- attention ----
    # ctx^T pair-stacked: head h lives at partitions (h%2)*64, pair h//2
    ctxT = const.tile([P, HPC // 2, S], BF16, tag="ctxT")

    def attend_pair(j, hooks=None, carry=None):
        # Heads 2j (PE rows 0:64) and 2j+1 (64:128): scores bf16 per key
        # chunk; exp fp8e5m2 into kc-paired tiles (ACT exact for the even
        # head, Schraudolph uint8-bitcast on DVE for the odd); ctx is one
        # DoubleRow matmul per kc pair (contract 256 keys). Each quarter's
        # softmax renorm is DEFERRED into the next quarter's early
        # iterations (psC bufs=4 holds the two extra pinned ctx tiles).
        hooks = dict(hooks or {})
        h0, h1 = 2 * j, 2 * j + 1

        def make_renorm(qh, ctx0, ctx1):
            col = qh * QHN
            st = {}

            def gather_step():
                # both heads' exp-sums rows: aligned DVE copies out of PSUM,
                # then an SBUF->SBUF DMA shifts them to partitions 0/1
                sums = small.tile(
                    [HD + 1, 2, QHN], BF16, tag="sums", name=f"sm{j}{qh}"
                )
                nc.vector.tensor_copy(
                    sums[HD : HD + 1, 0, :], ctx0[HD : HD + 1, :]
                )
                nc.vector.tensor_copy(
                    sums[HD : HD + 1, 1, :], ctx1[HD : HD + 1, :]
                )
                zq = small.tile([2, QHN], BF16, tag="zq", name=f"zq{j}{qh}")
                nc.sync.dma_start(zq[:], sums[HD : HD + 1, :, :])
                st["zq"] = zq

            def bcast_step():
                # selector matmuls broadcast Z0/Z1 into the pinned ctx
                # tiles' unused partitions 64:128 (fp32 PE, idle slack) —
                # costs no extra PSUM bank; the reciprocal then doubles as
                # the PSUM->SBUF copy
                nc.tensor.matmul(
                    ctx0[HD:P, :], pat2[:, 0:HD], st["zq"][:],
                    start=True, stop=True,
                )
                nc.tensor.matmul(
                    ctx1[HD:P, :], pat2[:, HD:P], st["zq"][:],
                    start=True, stop=True,
                )

            def recip_step():
                zb = small.tile([HD, 2, QHN], F32, tag="zb", name=f"zb{j}{qh}")
                nc.vector.reciprocal(zb[:, 0, :], ctx0[HD:P, :])
                nc.vector.reciprocal(zb[:, 1, :], ctx1[HD:P, :])
                st["zb"] = zb

            def mult_step(hh, cps):
                po = (hh % 2) * HD
                nc.vector.tensor_tensor(
                    ctxT[po : po + HD, j, col : col + QHN],
                    cps[:HD, :],
                    st["zb"][:, hh % 2, :],
                    ALU.mult,
                )

            return [
                gather_step,
                bcast_step,
                recip_step,
                lambda: (mult_step(h0, ctx0), mult_step(h1, ctx1)),
            ]

        pending = list(carry) if carry else []
        for qh in range(S // QHN):
            col = qh * QHN
            ctx0 = psC.tile([P, QHN], F32, tag="ctx", name=f"c0_{j}_{qh}")
            ctx1 = psC.tile([P, QHN], F32, tag="ctx", name=f"c1_{j}_{qh}")
            for kcp in range(NKC // 2):
                ex0 = expp.tile([P, 2, QHN], U8, tag="ex", name=f"e0_{j}{qh}{kcp}")
                ex1u = expp.tile([P, 2, QHN], U8, tag="ex", name=f"e1_{j}{qh}{kcp}")
                for sub in range(2):
                    kc = 2 * kcp + sub
                    kcs = slice(kc * P, (kc + 1) * P)
                    ps0 = psS.tile([P, QHN], F32, tag="ps")
                    ps1 = psS.tile([P, QHN], F32, tag="ps")
                    nc.tensor.matmul(
                        ps0[:], KT[0:HD, j, kcs], QT[0:HD, j, col : col + QHN],
                        start=True, stop=True,
                    )
                    nc.tensor.matmul(
                        ps1[:], KT[HD:P, j, kcs], QT[HD:P, j, col : col + QHN],
                        start=True, stop=True,
                    )
                    nc.scalar.activation(
                        ex0[:, sub, :].bitcast(E5), ps0[:], AF.Exp, scale=SCL
                    )
                    if kc in (5, 7, 9, 11):
                        # DVE carries the renorm burst at kc 5..9; route the
                        # odd head's exp through ScalarE there
                        nc.scalar.activation(
                            ex1u[:, sub, :].bitcast(E5), ps1[:], AF.Exp,
                            scale=SCL,
                        )
                    else:
                        nc.vector.tensor_scalar(
                            out=ex1u[:, sub, :], in0=ps1[:],
                            scalar1=SCH_A5, scalar2=SCH_B5,
                            op0=ALU.mult, op1=ALU.add,
                        )
                    hook = hooks.pop((qh, kc), None)
                    if hook is not None:
                        hook()
                    if kc in (3, 5, 7, 9) and pending:
                        pending.pop(0)()
                nc.tensor.matmul(
                    ctx0[:], v8[:, 2 * kcp : 2 * kcp + 2, h0, :],
                    ex0[:].bitcast(E5),
                    start=(kcp == 0), stop=(kcp == NKC // 2 - 1),
                    perf_mode=DR,
                )
                nc.tensor.matmul(
                    ctx1[:], v8[:, 2 * kcp : 2 * kcp + 2, h1, :],
                    ex1u[:].bitcast(E5),
                    start=(kcp == 0), stop=(kcp == NKC // 2 - 1),
                    perf_mode=DR,
                )
            for fn in pending:
                fn()
            pending = make_renorm(qh, ctx0, ctx1)
        return pending

    # ---- AllToAll staging: exchange per-head-pair ctx so each rank holds
    # all 16 heads for its own 512-token shard. Cross-batch-group chunks are
    # zeroed via the host-provided group mask and summed away on receive.
    def stage_chunks(jj, a_in, quarters, eng=None):
        # pair-0 staging runs on Pool (idle until A2A#0, and ordered before
        # it in the Pool stream); pair-1 staging stays on DVE (Pool is
        # mid-collective then)
        eng = eng or nc.vector
        for q in quarters:
            for d in (q, q + RANKS):
                a2aS = a2ap.tile([P, SSH], E4, tag="a2aS", name=f"a2aS{jj}_{d}")
                eng.tensor_scalar_mul(
                    a2aS[:],
                    ctxT[:, jj, q * SSH : (q + 1) * SSH],
                    maskb[:, d : d + 1],
                )
                nc.sync.dma_start(a_in[d], a2aS[:])

    def trigger_a2a(a_in, a_out):
        if no_collective:
            nc.sync.dma_start(a_out[:], a_in[:])
        else:
            nc.gpsimd.collective_compute(
                "AllToAll",
                ALU.bypass,
                replica_groups=[[c for g in GROUPS for c in g]],
                ins=[a_in[:].opt()],
                outs=[a_out[:].opt()],
            )

    carry0 = attend_pair(0)

    # residual + out-bias staged during pair-1 (SP/Pool have slack there)
    xpb = const.tile([P, SSH // P, H], F32, tag="xpb")
    for tj in range(SSH // P):
        xr = epi.tile([P, H], F32, tag="xr")
        nc.gpsimd.dma_start(xr[:], xres[tj * P : (tj + 1) * P, :])
        nc.gpsimd.tensor_tensor(xpb[:, tj, :], xr[:], bo_bc[:], ALU.add)
    p1_hooks = {}
    # A2A#0 staged+fired inside pair 1 once the carried pair-0 renorm drains
    p1_hooks[(0, 11)] = lambda: stage_chunks(0, a2a_in0, [0, 1], nc.gpsimd)
    p1_hooks[(0, 13)] = lambda: (
        stage_chunks(0, a2a_in0, [2, 3], nc.gpsimd),
        trigger_a2a(a2a_in0, a2a_out0),
    )
    # A2A#1 chunks staged per quarter as their deferred renorms complete
    p1_hooks[(1, 11)] = lambda: stage_chunks(1, a2a_in1, [0])
    p1_hooks[(2, 11)] = lambda: stage_chunks(1, a2a_in1, [1])
    p1_hooks[(3, 11)] = lambda: stage_chunks(1, a2a_in1, [2])
    carry1 = attend_pair(1, p1_hooks, carry=carry0)
    for fn in carry1:
        fn()
    stage_chunks(1, a2a_in1, [3])
    trigger_a2a(a2a_in1, a2a_out1)

    # all-16-head pair-stacked ctx^T for my token shard: block = global
    # head pair src*2+jj. Chunk src and src+4 carry the two batch groups'
    # copies (one zeroed by the mask) — sum them away on Pool (idle between
    # the collectives). Even pairs arrive with A2A#0; their out-proj phase
    # accumulates into pinned PSUM while A2A#1 is on the wire, and the odd
    # phase continues the same accumulation groups after it lands.
    remT8 = const.tile([P, NH // 2, SSH], E4, tag="remT8")

    def recv_pairs(jj, a_out):
        # batched receive: two strided DMAs pull all four sources' chunks
        # (both group copies), then two wide adds mask-sum them into the
        # pair-stacked remT8 blocks. jj=0 lands on DVE (Pool is mid-A2A#1),
        # jj=1 on Pool.
        eng = nc.vector if jj == 0 else nc.gpsimd
        raA = a2ap.tile([P, RANKS, SSH], E4, tag="a2aRa", name=f"ra{jj}")
        rbA = a2ap.tile([P, RANKS, SSH], E4, tag="a2aRb", name=f"rb{jj}")
        dq = nc.scalar if jj == 0 else nc.sync
        dq.dma_start(raA[:], a_out[0:RANKS].rearrange("s p f -> p s f"))
        dq.dma_start(rbA[:], a_out[RANKS:].rearrange("s p f -> p s f"))
        for h in range(2):
            eng.tensor_tensor(
                remT8[:, jj + 4 * h : jj + 4 * h + 3 : 2, :],
                raA[:, 2 * h : 2 * h + 2, :],
                rbA[:, 2 * h : 2 * h + 2, :],
                ALU.add,
            )

    with tc.tile_wait_until(RECV0_WAIT_MS):
        recv_pairs(0, a2a_out0)

    def outproj_phase(jj, dst_fn):
        # DoubleRow-pair the 4 global head pairs {jj, jj+2, jj+4, jj+6}
        for tj in range(SSH // P):
            pso = [
                psS.tile([P, 512], F32, tag="ps", name=f"op{jj}_{tj}_{ncn}")
                for ncn in range(2)
            ]
            for ncn in range(2):
                for a in range(2):
                    sel = slice(jj + 4 * a, jj + 4 * a + 3, 2)
                    nc.tensor.matmul(
                        pso[ncn][:],
                        remT8[:, sel, tj * P : (tj + 1) * P],
                        wo8[:, sel, ncn * 512 : (ncn + 1) * 512],
                        start=(a == 0),
                        stop=(a == 1),
                        perf_mode=DR,
                    )
            dst_fn(tj, pso)

    # even pairs fold into xpb while A2A#1 is on the wire (DVE is idle there)
    def fold_xpb(tj, pso):
        for ncn in range(2):
            nc.vector.tensor_tensor(
                xpb[:, tj, ncn * 512 : (ncn + 1) * 512], pso[ncn][:],
                xpb[:, tj, ncn * 512 : (ncn + 1) * 512], ALU.add,
            )

    outproj_phase(0, fold_xpb)
    recv_pairs(1, a2a_out1)

    ys = []

    def fold_y(tj, pso):
        y = epi.tile([P, H], F32, tag="y", name=f"y{tj}")
        for ncn in range(2):
            nc.vector.tensor_tensor(
                y[:, ncn * 512 : (ncn + 1) * 512], pso[ncn][:],
                xpb[:, tj, ncn * 512 : (ncn + 1) * 512], ALU.add,
            )
        ys.append(y)

    outproj_phase(1, fold_y)

    # ---- residual + LayerNorm on the shard ----
    for tj in range(SSH // P):
        y = ys[tj]
        stats = small.tile([P, 2, 6], F32, tag="stats")
        for sg in range(2):
            nc.vector.bn_stats(
                stats[:, sg, :], y[:].rearrange("p (s f) -> p s f", s=2)[:, sg, :]
            )
        mv = small.tile([P, 4], F32, tag="mv")
        nc.vector.bn_aggr(mv[:, 0:2], stats[:])
        nc.scalar.activation(
            mv[:, 1:2], mv[:, 1:2], AF.Sqrt, bias=eps_sb[:], scale=1.0
        )
        nc.vector.reciprocal(mv[:, 1:2], mv[:, 1:2])
        # -mu/sigma, then y_n = y*(1/sigma) + (-mu/sigma) on ACT
        nc.vector.scalar_tensor_tensor(
            out=mv[:, 2:3], in0=mv[:, 0:1], scalar=-1.0, in1=mv[:, 1:2],
            op0=ALU.mult, op1=ALU.mult,
        )
        nc.scalar.activation(
            y[:], y[:], AF.Identity, bias=mv[:, 2:3], scale=mv[:, 1:2]
        )
        geng = nc.gpsimd if tj % 2 == 0 else nc.vector
        geng.tensor_tensor(y[:], y[:], gamma_bc[:], ALU.mult)
        geng.tensor_tensor(y[:], y[:], beta_bc[:], ALU.add)
        nc.sync.dma_start(out_d[tj * P : (tj + 1) * P, : H // 2], y[:, : H // 2])
        nc.scalar.dma_start(out_d[tj * P : (tj + 1) * P, H // 2 :], y[:, H // 2 :])


_NC_CACHE = None


def _get_nc():
    global _NC_CACHE
    if _NC_CACHE is None:
        _NC_CACHE = build()
    return _NC_CACHE


class Runner:
    """Compile once, execute many times via PJRT (mirrors
    bass2jax.run_bass_via_pjrt but keeps the jitted executable and device
    buffers so repeated calls measure steady-state device time)."""

    def __init__(self):
        import jax
        from jax.sharding import Mesh, PartitionSpec
        from jax.experimental.shard_map import shard_map
        from concourse import bass2jax, mybir as _mb

        bass2jax.install_neuronx_cc_hook()
        nc = _get_nc()
        self.nc = nc
        partition_name = (
            nc.partition_id_tensor.name if nc.partition_id_tensor else None
        )
        in_names, out_names, out_avals, zero_outs = [], [], [], []
        for alloc in nc.m.functions[0].allocations:
            if not isinstance(alloc, _mb.MemoryLocationSet):
                continue
            name = alloc.memorylocations[0].name
            if alloc.kind == "ExternalInput":
                if name != partition_name:
                    in_names.append(name)
            elif alloc.kind == "ExternalOutput":
                shape = tuple(alloc.tensor_shape)
                dtype = _mb.dt.np(alloc.dtype)
                out_names.append(name)
                out_avals.append(jax.core.ShapedArray(shape, dtype))
                zero_outs.append(np.zeros(shape, dtype))
        self.in_names, self.out_names = in_names, out_names
        self.zero_outs = zero_outs
        n_params, n_outs = len(in_names), len(out_names)
        all_names = in_names + out_names
        if partition_name is not None:
            all_names = all_names + [partition_name]
        donate = tuple(range(n_params, n_params + n_outs))

        def _body(*args):
            operands = list(args)
            if partition_name is not None:
                operands.append(bass2jax.partition_id_tensor())
            outs = bass2jax._bass_exec_p.bind(
                *operands,
                out_avals=tuple(out_avals),
                in_names=tuple(all_names),
                out_names=tuple(out_names),
                lowering_input_output_aliases=(),
                sim_require_finite=True,
                sim_require_nnan=True,
                nc=nc,
            )
            return tuple(outs)

        devices = jax.devices()[:N_CORES]
        self.mesh = Mesh(np.asarray(devices), ("core",))
        in_specs = (PartitionSpec("core"),) * (n_params + n_outs)
        out_specs = (PartitionSpec("core"),) * n_outs
        self.sharded = jax.jit(
            shard_map(
                _body,
                mesh=self.mesh,
                in_specs=in_specs,
                out_specs=out_specs,
                check_rep=False,
            ),
            donate_argnums=donate,
            keep_unused=True,
        )
        self._jax = jax

    def device_inputs(self, in_maps):
        import jax
        from jax.sharding import NamedSharding, PartitionSpec

        sh = NamedSharding(self.mesh, PartitionSpec("core"))
        args = []
        for name in self.in_names:
            cat = np.concatenate([np.asarray(m[name]) for m in in_maps], axis=0)
            args.append(jax.device_put(cat, sh))
        outs = [
            jax.device_put(np.concatenate([z] * N_CORES, axis=0), sh)
            for z in self.zero_outs
        ]
        return args, outs

    def run(self, in_maps):
        args, outs = self.device_inputs(in_maps)
        res = self.sharded(*args, *outs)
        per_core = []
        for c in range(N_CORES):
            d = {}
            for i, name in enumerate(self.out_names):
                full = np.asarray(res[i])
                n0 = full.shape[0] // N_CORES
                d[name] = full[c * n0 : (c + 1) * n0]
            per_core.append(d)
        return per_core

    def time_exec(self, in_maps, iters=20, warmup=3):
        return self.time_exec_windows(in_maps, iters=iters, warmup=warmup)[0]

    def time_exec_windows(
        self, in_maps, iters=20, warmup=3, windows=1, gap_s=0.0
    ):
        """Time `windows` independent iters-long windows sharing one set of
        device buffers; returns per-window ns/iter. The axon tunnel adds
        large time-varying congestion noise, so callers take the min."""
        import time

        args, outs = self.device_inputs(in_maps)
        vals = []
        for w in range(windows):
            for _ in range(warmup):
                res = self.sharded(*args, *outs)
                outs = list(res)
            self._jax.block_until_ready(outs)
            t0 = time.perf_counter()
            for _ in range(iters):
                res = self.sharded(*args, *outs)
                outs = list(res)
            self._jax.block_until_ready(outs)
            t1 = time.perf_counter()
            vals.append((t1 - t0) / iters)
            if gap_s and w < windows - 1:
                time.sleep(gap_s)
        return vals


_RUNNER = None


def _get_runner():
    global _RUNNER
    if _RUNNER is None:
        _RUNNER = Runner()
    return _RUNNER


def make_in_maps(inputs):
    import ml_dtypes

    e4 = ml_dtypes.float8_e4m3
    x = np.asarray(inputs["x"], np.float32)
    wq, wk, wv = (np.asarray(inputs[k], np.float32) for k in ("Wq", "Wk", "Wv"))
    wo = np.asarray(inputs["Wo"], np.float32)
    bq, bk, bv = (np.asarray(inputs[k], np.float32) for k in ("bq", "bk", "bv"))
    bo = np.asarray(inputs["bo"], np.float32)
    gamma = np.asarray(inputs["ln_gamma"], np.float32)
    beta = np.asarray(inputs["ln_beta"], np.float32)

    # x[g]^T in fp8e4m3, laid out [P, KO, S]
    x8g = [
        np.ascontiguousarray(
            x[g].T.reshape(KO, P, S).transpose(1, 0, 2)
        ).astype(e4)
        for g in range(B)
    ]
    wo8 = np.ascontiguousarray(
        wo.reshape(NH // 2, P, H).transpose(1, 0, 2)
    ).astype(e4)

    def wslice(w, cols):
        return np.ascontiguousarray(
            w[:, cols].reshape(KO, P, DLOC).transpose(1, 0, 2)
        ).astype(e4)

    import ml_dtypes as _mld
    pat2_host = np.zeros((2, P), _mld.bfloat16)
    pat2_host[0, :HD] = 1.0
    pat2_host[1, HD:] = 1.0

    in_maps = []
    for c in range(N_CORES):
        g, r = c // RANKS, c % RANKS
        cols = slice(DLOC * r, DLOC * (r + 1))
        in_maps.append(
            {
                "x8": x8g[g],
                "xres": np.ascontiguousarray(x[g, SSH * r : SSH * (r + 1)]),
                "wq": wslice(wq, cols),
                "wk": wslice(wk, cols),
                "wv": wslice(wv, cols),
                "wo": wo8,
                "bq": np.ascontiguousarray(bq[cols]),
                "bk": np.ascontiguousarray(bk[cols]),
                "bv": np.ascontiguousarray(bv[cols]),
                "bo": bo,
                "gamma": gamma,
                "beta": beta,
                "mask": (np.arange(N_CORES) // RANKS == g).astype(np.float32),
                "pat2": pat2_host,
            }
        )
    return in_maps


def run_spmd(inputs, trace=False):
    results = _get_runner().run(make_in_maps(inputs))
    out = np.empty((B, S, H), np.float32)
    for c in range(N_CORES):
        g, r = c // RANKS, c % RANKS
        out[g, SSH * r : SSH * (r + 1)] = results[c]["out"]
    return out, results


def kernel(**inputs) -> np.ndarray:
    out, _ = run_spmd(inputs)
    return out


# revision 40
# speedup vs baseline: 1.0128x; 1.0128x over previous
"""Distributed Bass kernel for nn_AttentionLayer (B=2, S=2048, H=1024, NH=16).

Sharding: 8 cores = 2 batch groups x 4 ranks. Core c handles batch c//4 and
heads [4r:4r+4] (r = c%4). QKV projections are column-sharded; attention runs
per-head with a transposed dataflow (scores^T so softmax's reduction axis sits
on PSUM partitions and feeds the ctx matmul directly). The rank exchange moves
ctx (1 MB via AllToAll) instead of out-proj partials; each rank then computes
the full 16-head output projection + residual + LayerNorm on its 512-token
shard.

v2: all heavy matmuls except QK^T run as fp8e4m3 DoubleRow (contract 256 per
pass -> 2x PE): QKV projections pair k-tiles, ctx pairs key chunks, out-proj
pairs head pairs. x / Wq/Wk/Wv/Wo are pre-transposed + pre-quantized to fp8 on
the host, which also kills the on-device transposes and staging copies.
Softmax stays unnormalized-exp + deferred renorm (ones-column in V gives exp
sums as PSUM row 64); exp emits fp8e5m2 (ACT exact-exp for the even head,
uint8 Schraudolph bitcast on DVE for the odd head) feeding the DoubleRow ctx
matmul directly. QK^T stays bf16 (HD=64 contract gains nothing from fp8).
Attention is exp-bound (ACT+DVE saturated); everything else is scheduled
around that: the renorm's Z-broadcast goes through a tiny selector matmul
into the pinned ctx tiles' unused partitions (no Pool dependency — Pool
stalls behind in-flight collectives), SBUF-only work (pair-0 staging, xpb,
recv1 sums) rides Pool, and recv0's collective-blocked DMAs carry a
scheduler wait floor + run on the ACT queue so they can't wedge the SP
stream mid-attention. The second AllToAll fires immediately after the last
quarter's renorm; the even-pair out-proj phase + residual prep fill its
window, so only recv1+odd-phase+LN sit after it.
"""

import sys
from contextlib import ExitStack

sys.path.insert(0, "/opt/trn_rl_repo")

import numpy as np
from concourse import bacc, bass, bass_utils, mybir, tile

AF = mybir.ActivationFunctionType
ALU = mybir.AluOpType
DR = mybir.MatmulPerfMode.DoubleRow
F32 = mybir.dt.float32
BF16 = mybir.dt.bfloat16
E4 = mybir.dt.float8e4
E5 = mybir.dt.float8e5
U8 = mybir.dt.uint8

B, S, H, NH, HD = 2, 2048, 1024, 16, 64
N_CORES = 8
RANKS = 4  # ranks per batch group
GROUPS = [[0, 1, 2, 3], [4, 5, 6, 7]]
HPC = NH // RANKS  # heads per core = 4
DLOC = HPC * HD  # local head dims = 256
SSH = S // RANKS  # token shard = 512
LN_EPS = 1e-5
P = 128
KO = H // P  # 8 k-tiles over hidden dim
TI = S // P  # 16 token tiles
QHN = 512  # attention query-quarter width
NKC = S // P  # 16 key chunks
SCL = 0.125  # 1/sqrt(HD)
SCH_A5 = (4.0 / np.log(2.0)) * SCL  # Schraudolph slope for e5m2 (scale folded)
SCH_B5 = 60.0 - 0.22  # e5m2 offset, C tuned for RN convert
RECV0_WAIT_MS = 0.130  # scheduler floor for recv0 (keeps its collective-
                       # blocked DMAs out of the attention SP stream)


def build(no_collective=False):
    nc = bacc.Bacc("TRN2", target_bir_lowering=False, debug=False, num_devices=N_CORES)

    x8_d = nc.dram_tensor("x8", [P, KO, S], E4, kind="ExternalInput")
    xres = nc.dram_tensor("xres", [SSH, H], F32, kind="ExternalInput")
    mask_d = nc.dram_tensor("mask", [N_CORES], F32, kind="ExternalInput")
    wq_d = nc.dram_tensor("wq", [P, KO, DLOC], E4, kind="ExternalInput")
    wk_d = nc.dram_tensor("wk", [P, KO, DLOC], E4, kind="ExternalInput")
    wv_d = nc.dram_tensor("wv", [P, KO, DLOC], E4, kind="ExternalInput")
    wo_d = nc.dram_tensor("wo", [P, NH // 2, H], E4, kind="ExternalInput")
    bq_d = nc.dram_tensor("bq", [DLOC], F32, kind="ExternalInput")
    bk_d = nc.dram_tensor("bk", [DLOC], F32, kind="ExternalInput")
    bv_d = nc.dram_tensor("bv", [DLOC], F32, kind="ExternalInput")
    bo_d = nc.dram_tensor("bo", [H], F32, kind="ExternalInput")
    gamma_d = nc.dram_tensor("gamma", [H], F32, kind="ExternalInput")
    beta_d = nc.dram_tensor("beta", [H], F32, kind="ExternalInput")
    pat2_d = nc.dram_tensor("pat2", [HD, P], BF16, kind="ExternalInput")
    out_d = nc.dram_tensor("out", [SSH, H], F32, kind="ExternalOutput")

    with tile.TileContext(nc) as tc, ExitStack() as ctx:
        _build_body(
            nc, tc, ctx,
            x8_d, xres, mask_d, wq_d, wk_d, wv_d, wo_d, bq_d, bk_d, bv_d, bo_d,
            gamma_d, beta_d, pat2_d, out_d, no_collective=no_collective,
        )
    nc.compile()
    return nc


def _build_body(
    nc, tc, ctx, x8_d, xres, mask_d, wq_d, wk_d, wv_d, wo_d, bq_d, bk_d, bv_d,
    bo_d, gamma_d, beta_d, pat2_d, out_d, no_collective=False,
):
    const = ctx.enter_context(tc.tile_pool(name="const", bufs=1))
    stg = ctx.enter_context(tc.tile_pool(name="stg", bufs=2))
    expp = ctx.enter_context(tc.tile_pool(name="expp", bufs=6))
    small = ctx.enter_context(tc.tile_pool(name="small", bufs=2))
    epi = ctx.enter_context(tc.tile_pool(name="epi", bufs=2))
    dram = ctx.enter_context(tc.tile_pool(name="dram", bufs=1, space="DRAM"))
    psS = ctx.enter_context(tc.tile_pool(name="psS", bufs=4, space="PSUM"))
    psC = ctx.enter_context(tc.tile_pool(name="psC", bufs=4, space="PSUM"))
    a2ap = ctx.enter_context(tc.tile_pool(name="a2ap", bufs=3))

    a2a_in0 = dram.tile([N_CORES, P, SSH], E4, tag="a2a_in0")
    a2a_out0 = dram.tile([N_CORES, P, SSH], E4, tag="a2a_out0")
    a2a_in1 = dram.tile([N_CORES, P, SSH], E4, tag="a2a_in1")
    a2a_out1 = dram.tile([N_CORES, P, SSH], E4, tag="a2a_out1")

    # ---- front DMAs, spread across engine queues ----
    w8q = const.tile([P, KO, DLOC], E4, tag="w8q")
    w8k = const.tile([P, KO, DLOC], E4, tag="w8k")
    w8v = const.tile([P, KO, DLOC], E4, tag="w8v")
    nc.sync.dma_start(w8q[:], wq_d[:])
    nc.sync.dma_start(w8k[:], wk_d[:])
    nc.sync.dma_start(w8v[:], wv_d[:])

    x8 = const.tile([P, KO, S], E4, tag="x8")
    for c2 in range(0, KO, 2):
        eng = nc.sync if c2 < KO // 2 else nc.scalar
        eng.dma_start(x8[:, c2 : c2 + 2], x8_d[:, c2 : c2 + 2])

    wo8 = const.tile([P, NH // 2, H], E4, tag="wo8")
    for c4 in range(0, NH // 2, 4):
        nc.gpsimd.dma_start(wo8[:, c4 : c4 + 4], wo_d[:, c4 : c4 + 4])

    # per-partition biases for Q/K projections: [DLOC] -> [P, 2]
    bq_sb = const.tile([P, DLOC // P], F32)
    nc.sync.dma_start(bq_sb[:], bq_d[:].rearrange("(o p) -> p o", p=P))
    bk_sb = const.tile([P, DLOC // P], F32)
    nc.sync.dma_start(bk_sb[:], bk_d[:].rearrange("(o p) -> p o", p=P))

    # free-axis vectors, replicated across partitions via gpsimd
    def bcast_vec(dram_t, n):
        row = stg.tile([1, n], F32, tag="wstg")
        nc.sync.dma_start(row[:], dram_t[:].rearrange("(o n) -> o n", o=1))
        bc = const.tile([P, n], F32, tag=f"bc_{dram_t.name}")
        nc.gpsimd.partition_broadcast(bc[:], row[:])
        return bc

    bv_bc = bcast_vec(bv_d, DLOC)
    bo_bc = bcast_vec(bo_d, H)
    gamma_bc = bcast_vec(gamma_d, H)
    beta_bc = bcast_vec(beta_d, H)
    eps_sb = const.tile([P, 1], F32)
    nc.vector.memset(eps_sb[:], LN_EPS)

    maskb = const.tile([P, N_CORES], F32)
    mrow = stg.tile([1, N_CORES], F32, tag="wstg")
    nc.sync.dma_start(mrow[:], mask_d[:].rearrange("(o n) -> o n", o=1))
    nc.gpsimd.partition_broadcast(maskb[:], mrow[:])

    # renorm broadcast selector (64-partition operands, PE tile_size 64
    # exact): Z rows live at partitions 0 and 32 so plain DVE copies out
    # of PSUM can land them directly -- no SBUF->SBUF shift DMA
    pat2 = const.tile([HD, P], BF16, tag="pat2")
    nc.sync.dma_start(pat2[:], pat2_d[:])
    zq = const.tile([HD, QHN], BF16, tag="zq")
    nc.vector.memset(zq[:], 0.0)

    # ---- V in fp8e4m3 with a ones column at HD (exp-sums trick) ----
    v8 = const.tile([P, TI, HPC, P], E4)
    nc.gpsimd.memset(v8[:, :, :, HD], 1.0)
    nc.gpsimd.memset(v8[:, :, :, HD + 1 :], 0.0)

    # ---- projections (fp8 DoubleRow: contract 256 per matmul) ----
    QT = const.tile([P, DLOC // P, S], BF16)
    KT = const.tile([P, DLOC // P, S], BF16)

    def qk_proj(dst, w_sb, b_sb, pr, q4, alt):
        col = q4 * QHN
        ps = psS.tile([P, QHN], F32, tag="ps", name=f"qk{pr}_{q4}_{id(dst) % 97}")
        for kop in range(KO // 2):
            nc.tensor.matmul(
                ps[:],
                w_sb[:, 2 * kop : 2 * kop + 2, pr * P : (pr + 1) * P],
                x8[:, 2 * kop : 2 * kop + 2, col : col + QHN],
                start=(kop == 0),
                stop=(kop == KO // 2 - 1),
                perf_mode=DR,
            )
        # bias + bf16 convert, alternating ACT/DVE so neither gates the front
        if alt % 2 == 0:
            nc.scalar.activation(
                dst[:, pr, col : col + QHN], ps[:], AF.Identity,
                bias=b_sb[:, pr : pr + 1], scale=1.0,
            )
        else:
            nc.vector.tensor_scalar(
                out=dst[:, pr, col : col + QHN], in0=ps[:],
                scalar1=b_sb[:, pr : pr + 1], scalar2=None, op0=ALU.add,
            )

    def v_build(ti):
        ps = psS.tile([P, DLOC], F32, tag="ps", name=f"vb{ti}")
        for kop in range(KO // 2):
            nc.tensor.matmul(
                ps[:, :DLOC],
                x8[:, 2 * kop : 2 * kop + 2, ti * P : (ti + 1) * P],
                w8v[:, 2 * kop : 2 * kop + 2, :],
                start=(kop == 0),
                stop=(kop == KO // 2 - 1),
                perf_mode=DR,
            )
        nc.vector.tensor_tensor(
            v8[:, ti, :, :HD],
            ps[:, :DLOC].rearrange("p (h d) -> p h d", h=HPC),
            bv_bc[:].rearrange("p (h d) -> p h d", h=HPC),
            ALU.add,
        )

    for q4 in range(4):
        qk_proj(QT, w8q, bq_sb, 0, q4, 2 * q4)
        qk_proj(KT, w8k, bk_sb, 0, q4, 2 * q4 + 1)
    for ti in range(TI):
        v_build(ti)
    for q4 in range(4):
        qk_proj(QT, w8q, bq_sb, 1, q4, 2 * q4)
        qk_proj(KT, w8k, bk_sb, 1, q4, 2 * q4 + 1)

    # ---- attention ----
    # ctx^T pair-stacked: head h lives at partitions (h%2)*64, pair h//2
    ctxT = const.tile([P, HPC // 2, S], BF16, tag="ctxT")

    def attend_pair(j, hooks=None, carry=None):
        # Heads 2j (PE rows 0:64) and 2j+1 (64:128): scores bf16 per key
        # chunk; exp fp8e5m2 into kc-paired tiles (ACT exact for the even
        # head, Schraudolph uint8-bitcast on DVE for the odd); ctx is one
        # DoubleRow matmul per kc pair (contract 256 keys). Each quarter's
        # softmax renorm is DEFERRED into the next quarter's early
        # iterations (psC bufs=4 holds the two extra pinned ctx tiles).
        hooks = dict(hooks or {})
        h0, h1 = 2 * j, 2 * j + 1

        def make_renorm(qh, ctx0, ctx1):
            col = qh * QHN
            st = {}

            def gather_step():
                # both heads' exp-sums rows, copied straight from PSUM row 64
                # to zq partitions 0 / 32 (base-shifted, same class as the
                # hardware-validated recip_step PSUM reads)
                nc.vector.tensor_copy(zq[0:1, :], ctx0[HD : HD + 1, :])
                nc.vector.tensor_copy(zq[32:33, :], ctx1[HD : HD + 1, :])

            def bcast_step():
                # selector matmuls broadcast Z0/Z1 into the pinned ctx
                # tiles' unused partitions 64:128 (fp32 PE, idle slack) —
                # costs no extra PSUM bank; the reciprocal then doubles as
                # the PSUM->SBUF copy
                nc.tensor.matmul(
                    ctx0[HD:P, :], pat2[:, 0:HD], zq[:],
                    start=True, stop=True,
                )
                nc.tensor.matmul(
                    ctx1[HD:P, :], pat2[:, HD:P], zq[:],
                    start=True, stop=True,
                )

            def recip_step():
                zb = small.tile([HD, 2, QHN], F32, tag="zb", name=f"zb{j}{qh}")
                nc.vector.reciprocal(zb[:, 0, :], ctx0[HD:P, :])
                nc.vector.reciprocal(zb[:, 1, :], ctx1[HD:P, :])
                st["zb"] = zb

            def mult_step(hh, cps):
                po = (hh % 2) * HD
                nc.vector.tensor_tensor(
                    ctxT[po : po + HD, j, col : col + QHN],
                    cps[:HD, :],
                    st["zb"][:, hh % 2, :],
                    ALU.mult,
                )

            return [
                gather_step,
                bcast_step,
                recip_step,
                lambda: (mult_step(h0, ctx0), mult_step(h1, ctx1)),
            ]

        pending = list(carry) if carry else []
        for qh in range(S // QHN):
            col = qh * QHN
            ctx0 = psC.tile([P, QHN], F32, tag="ctx", name=f"c0_{j}_{qh}")
            ctx1 = psC.tile([P, QHN], F32, tag="ctx", name=f"c1_{j}_{qh}")
            for kcp in range(NKC // 2):
                ex0 = expp.tile([P, 2, QHN], U8, tag="ex", name=f"e0_{j}{qh}{kcp}")
                ex1u = expp.tile([P, 2, QHN], U8, tag="ex", name=f"e1_{j}{qh}{kcp}")
                for sub in range(2):
                    kc = 2 * kcp + sub
                    kcs = slice(kc * P, (kc + 1) * P)
                    ps0 = psS.tile([P, QHN], F32, tag="ps")
                    ps1 = psS.tile([P, QHN], F32, tag="ps")
                    nc.tensor.matmul(
                        ps0[:], KT[0:HD, j, kcs], QT[0:HD, j, col : col + QHN],
                        start=True, stop=True,
                    )
                    nc.tensor.matmul(
                        ps1[:], KT[HD:P, j, kcs], QT[HD:P, j, col : col + QHN],
                        start=True, stop=True,
                    )
                    nc.scalar.activation(
                        ex0[:, sub, :].bitcast(E5), ps0[:], AF.Exp, scale=SCL
                    )
                    if kc in (5, 7, 9, 11):
                        # DVE carries the renorm burst at kc 5..9; route the
                        # odd head's exp through ScalarE there
                        nc.scalar.activation(
                            ex1u[:, sub, :].bitcast(E5), ps1[:], AF.Exp,
                            scale=SCL,
                        )
                    else:
                        nc.vector.tensor_scalar(
                            out=ex1u[:, sub, :], in0=ps1[:],
                            scalar1=SCH_A5, scalar2=SCH_B5,
                            op0=ALU.mult, op1=ALU.add,
                        )
                    hook = hooks.pop((qh, kc), None)
                    if hook is not None:
                        hook()
                    if kc in (3, 5, 7, 9) and pending:
                        pending.pop(0)()
                nc.tensor.matmul(
                    ctx0[:], v8[:, 2 * kcp : 2 * kcp + 2, h0, :],
                    ex0[:].bitcast(E5),
                    start=(kcp == 0), stop=(kcp == NKC // 2 - 1),
                    perf_mode=DR,
                )
                nc.tensor.matmul(
                    ctx1[:], v8[:, 2 * kcp : 2 * kcp + 2, h1, :],
                    ex1u[:].bitcast(E5),
                    start=(kcp == 0), stop=(kcp == NKC // 2 - 1),
                    perf_mode=DR,
                )
            for fn in pending:
                fn()
            pending = make_renorm(qh, ctx0, ctx1)
        return pending

    # ---- AllToAll staging: exchange per-head-pair ctx so each rank holds
    # all 16 heads for its own 512-token shard. Cross-batch-group chunks are
    # zeroed via the host-provided group mask and summed away on receive.
    def stage_chunks(jj, a_in, quarters, eng=None):
        # pair-0 staging runs on Pool (idle until A2A#0, and ordered before
        # it in the Pool stream); pair-1 staging stays on DVE (Pool is
        # mid-collective then)
        eng = eng or nc.vector
        for q in quarters:
            for d in (q, q + RANKS):
                a2aS = a2ap.tile([P, SSH], E4, tag="a2aS", name=f"a2aS{jj}_{d}")
                eng.tensor_scalar_mul(
                    a2aS[:],
                    ctxT[:, jj, q * SSH : (q + 1) * SSH],
                    maskb[:, d : d + 1],
                )
                nc.sync.dma_start(a_in[d], a2aS[:])

    def trigger_a2a(a_in, a_out):
        if no_collective:
            nc.sync.dma_start(a_out[:], a_in[:])
        else:
            nc.gpsimd.collective_compute(
                "AllToAll",
                ALU.bypass,
                replica_groups=[[c for g in GROUPS for c in g]],
                ins=[a_in[:].opt()],
                outs=[a_out[:].opt()],
            )

    carry0 = attend_pair(0)

    # residual + out-bias staged during pair-1 (SP/Pool have slack there)
    xpb = const.tile([P, SSH // P, H], F32, tag="xpb")
    for tj in range(SSH // P):
        xr = epi.tile([P, H], F32, tag="xr")
        nc.gpsimd.dma_start(xr[:], xres[tj * P : (tj + 1) * P, :])
        nc.gpsimd.tensor_tensor(xpb[:, tj, :], xr[:], bo_bc[:], ALU.add)
    p1_hooks = {}
    # A2A#0 staged+fired inside pair 1 once the carried pair-0 renorm drains
    p1_hooks[(0, 11)] = lambda: stage_chunks(0, a2a_in0, [0, 1], nc.gpsimd)
    p1_hooks[(0, 13)] = lambda: (
        stage_chunks(0, a2a_in0, [2, 3], nc.gpsimd),
        trigger_a2a(a2a_in0, a2a_out0),
    )
    # A2A#1 chunks staged per quarter as their deferred renorms complete
    p1_hooks[(1, 11)] = lambda: stage_chunks(1, a2a_in1, [0])
    p1_hooks[(2, 11)] = lambda: stage_chunks(1, a2a_in1, [1])
    p1_hooks[(3, 11)] = lambda: stage_chunks(1, a2a_in1, [2])
    carry1 = attend_pair(1, p1_hooks, carry=carry0)
    for fn in carry1:
        fn()
    stage_chunks(1, a2a_in1, [3])
    trigger_a2a(a2a_in1, a2a_out1)

    # all-16-head pair-stacked ctx^T for my token shard: block = global
    # head pair src*2+jj. Chunk src and src+4 carry the two batch groups'
    # copies (one zeroed by the mask) — sum them away on Pool (idle between
    # the collectives). Even pairs arrive with A2A#0; their out-proj phase
    # accumulates into pinned PSUM while A2A#1 is on the wire, and the odd
    # phase continues the same accumulation groups after it lands.
    remT8 = const.tile([P, NH // 2, SSH], E4, tag="remT8")

    def recv_pairs(jj, a_out):
        # batched receive: two strided DMAs pull all four sources' chunks
        # (both group copies), then two wide adds mask-sum them into the
        # pair-stacked remT8 blocks. jj=0 lands on DVE (Pool is mid-A2A#1),
        # jj=1 on Pool.
        eng = nc.vector if jj == 0 else nc.gpsimd
        raA = a2ap.tile([P, RANKS, SSH], E4, tag="a2aRa", name=f"ra{jj}")
        rbA = a2ap.tile([P, RANKS, SSH], E4, tag="a2aRb", name=f"rb{jj}")
        dq = nc.scalar if jj == 0 else nc.sync
        dq.dma_start(raA[:], a_out[0:RANKS].rearrange("s p f -> p s f"))
        dq.dma_start(rbA[:], a_out[RANKS:].rearrange("s p f -> p s f"))
        for h in range(2):
            eng.tensor_tensor(
                remT8[:, jj + 4 * h : jj + 4 * h + 3 : 2, :],
                raA[:, 2 * h : 2 * h + 2, :],
                rbA[:, 2 * h : 2 * h + 2, :],
                ALU.add,
            )

    with tc.tile_wait_until(RECV0_WAIT_MS):
        recv_pairs(0, a2a_out0)

    def outproj_phase(jj, dst_fn):
        # DoubleRow-pair the 4 global head pairs {jj, jj+2, jj+4, jj+6}
        for tj in range(SSH // P):
            pso = [
                psS.tile([P, 512], F32, tag="ps", name=f"op{jj}_{tj}_{ncn}")
                for ncn in range(2)
            ]
            for ncn in range(2):
                for a in range(2):
                    sel = slice(jj + 4 * a, jj + 4 * a + 3, 2)
                    nc.tensor.matmul(
                        pso[ncn][:],
                        remT8[:, sel, tj * P : (tj + 1) * P],
                        wo8[:, sel, ncn * 512 : (ncn + 1) * 512],
                        start=(a == 0),
                        stop=(a == 1),
                        perf_mode=DR,
                    )
            dst_fn(tj, pso)

    # even pairs fold into xpb while A2A#1 is on the wire (DVE is idle there)
    def fold_xpb(tj, pso):
        for ncn in range(2):
            nc.vector.tensor_tensor(
                xpb[:, tj, ncn * 512 : (ncn + 1) * 512], pso[ncn][:],
                xpb[:, tj, ncn * 512 : (ncn + 1) * 512], ALU.add,
            )

    outproj_phase(0, fold_xpb)
    recv_pairs(1, a2a_out1)

    ys = []

    def fold_y(tj, pso):
        y = epi.tile([P, H], F32, tag="y", name=f"y{tj}")
        for ncn in range(2):
            nc.vector.tensor_tensor(
                y[:, ncn * 512 : (ncn + 1) * 512], pso[ncn][:],
                xpb[:, tj, ncn * 512 : (ncn + 1) * 512], ALU.add,
            )
        ys.append(y)

    outproj_phase(1, fold_y)

    # ---- residual + LayerNorm on the shard ----
    for tj in range(SSH // P):
        y = ys[tj]
        stats = small.tile([P, 2, 6], F32, tag="stats")
        for sg in range(2):
            nc.vector.bn_stats(
                stats[:, sg, :], y[:].rearrange("p (s f) -> p s f", s=2)[:, sg, :]
            )
        mv = small.tile([P, 4], F32, tag="mv")
        nc.vector.bn_aggr(mv[:, 0:2], stats[:])
        nc.scalar.activation(
            mv[:, 1:2], mv[:, 1:2], AF.Sqrt, bias=eps_sb[:], scale=1.0
        )
        nc.vector.reciprocal(mv[:, 1:2], mv[:, 1:2])
        # -mu/sigma, then y_n = y*(1/sigma) + (-mu/sigma) on ACT
        nc.vector.scalar_tensor_tensor(
            out=mv[:, 2:3], in0=mv[:, 0:1], scalar=-1.0, in1=mv[:, 1:2],
            op0=ALU.mult, op1=ALU.mult,
        )
        nc.scalar.activation(
            y[:], y[:], AF.Identity, bias=mv[:, 2:3], scale=mv[:, 1:2]
        )
        geng = nc.gpsimd if tj % 2 == 0 else nc.vector
        geng.tensor_tensor(y[:], y[:], gamma_bc[:], ALU.mult)
        geng.tensor_tensor(y[:], y[:], beta_bc[:], ALU.add)
        nc.sync.dma_start(out_d[tj * P : (tj + 1) * P, : H // 2], y[:, : H // 2])
        nc.scalar.dma_start(out_d[tj * P : (tj + 1) * P, H // 2 :], y[:, H // 2 :])


_NC_CACHE = None


def _get_nc():
    global _NC_CACHE
    if _NC_CACHE is None:
        _NC_CACHE = build()
    return _NC_CACHE


class Runner:
    """Compile once, execute many times via PJRT (mirrors
    bass2jax.run_bass_via_pjrt but keeps the jitted executable and device
    buffers so repeated calls measure steady-state device time)."""

    def __init__(self):
        import jax
        from jax.sharding import Mesh, PartitionSpec
        from jax.experimental.shard_map import shard_map
        from concourse import bass2jax, mybir as _mb

        bass2jax.install_neuronx_cc_hook()
        nc = _get_nc()
        self.nc = nc
        partition_name = (
            nc.partition_id_tensor.name if nc.partition_id_tensor else None
        )
        in_names, out_names, out_avals, zero_outs = [], [], [], []
        for alloc in nc.m.functions[0].allocations:
            if not isinstance(alloc, _mb.MemoryLocationSet):
                continue
            name = alloc.memorylocations[0].name
            if alloc.kind == "ExternalInput":
                if name != partition_name:
                    in_names.append(name)
            elif alloc.kind == "ExternalOutput":
                shape = tuple(alloc.tensor_shape)
                dtype = _mb.dt.np(alloc.dtype)
                out_names.append(name)
                out_avals.append(jax.core.ShapedArray(shape, dtype))
                zero_outs.append(np.zeros(shape, dtype))
        self.in_names, self.out_names = in_names, out_names
        self.zero_outs = zero_outs
        n_params, n_outs = len(in_names), len(out_names)
        all_names = in_names + out_names
        if partition_name is not None:
            all_names = all_names + [partition_name]
        donate = tuple(range(n_params, n_params + n_outs))

        def _body(*args):
            operands = list(args)
            if partition_name is not None:
                operands.append(bass2jax.partition_id_tensor())
            outs = bass2jax._bass_exec_p.bind(
                *operands,
                out_avals=tuple(out_avals),
                in_names=tuple(all_names),
                out_names=tuple(out_names),
                lowering_input_output_aliases=(),
                sim_require_finite=True,
                sim_require_nnan=True,
                nc=nc,
            )
            return tuple(outs)

        devices = jax.devices()[:N_CORES]
        self.mesh = Mesh(np.asarray(devices), ("core",))
        in_specs = (PartitionSpec("core"),) * (n_params + n_outs)
        out_specs = (PartitionSpec("core"),) * n_outs
        self.sharded = jax.jit(
            shard_map(
                _body,
                mesh=self.mesh,
                in_specs=in_specs,
                out_specs=out_specs,
                check_rep=False,
            ),
            donate_argnums=donate,
            keep_unused=True,
        )
        self._jax = jax

    def device_inputs(self, in_maps):
        import jax
        from jax.sharding import NamedSharding, PartitionSpec

        sh = NamedSharding(self.mesh, PartitionSpec("core"))
        args = []
        for name in self.in_names:
            cat = np.concatenate([np.asarray(m[name]) for m in in_maps], axis=0)
            args.append(jax.device_put(cat, sh))
        outs = [
            jax.device_put(np.concatenate([z] * N_CORES, axis=0), sh)
            for z in self.zero_outs
        ]
        return args, outs

    def run(self, in_maps):
        args, outs = self.device_inputs(in_maps)
        res = self.sharded(*args, *outs)
        per_core = []
        for c in range(N_CORES):
            d = {}
            for i, name in enumerate(self.out_names):
                full = np.asarray(res[i])
                n0 = full.shape[0] // N_CORES
                d[name] = full[c * n0 : (c + 1) * n0]
            per_core.append(d)
        return per_core

    def time_exec(self, in_maps, iters=20, warmup=3):
        return self.time_exec_windows(in_maps, iters=iters, warmup=warmup)[0]

    def time_exec_windows(
        self, in_maps, iters=20, warmup=3, windows=1, gap_s=0.0
    ):
        """Time `windows` independent iters-long windows sharing one set of
        device buffers; returns per-window ns/iter. The axon tunnel adds
        large time-varying congestion noise, so callers take the min."""
        import time

        args, outs = self.device_inputs(in_maps)
        vals = []
        for w in range(windows):
            for _ in range(warmup):
                res = self.sharded(*args, *outs)
                outs = list(res)
            self._jax.block_until_ready(outs)
            t0 = time.perf_counter()
            for _ in range(iters):
                res = self.sharded(*args, *outs)
                outs = list(res)
            self._jax.block_until_ready(outs)
            t1 = time.perf_counter()
            vals.append((t1 - t0) / iters)
            if gap_s and w < windows - 1:
                time.sleep(gap_s)
        return vals


_RUNNER = None


def _get_runner():
    global _RUNNER
    if _RUNNER is None:
        _RUNNER = Runner()
    return _RUNNER


def make_in_maps(inputs):
    import ml_dtypes

    e4 = ml_dtypes.float8_e4m3
    x = np.asarray(inputs["x"], np.float32)
    wq, wk, wv = (np.asarray(inputs[k], np.float32) for k in ("Wq", "Wk", "Wv"))
    wo = np.asarray(inputs["Wo"], np.float32)
    bq, bk, bv = (np.asarray(inputs[k], np.float32) for k in ("bq", "bk", "bv"))
    bo = np.asarray(inputs["bo"], np.float32)
    gamma = np.asarray(inputs["ln_gamma"], np.float32)
    beta = np.asarray(inputs["ln_beta"], np.float32)

    # x[g]^T in fp8e4m3, laid out [P, KO, S]
    x8g = [
        np.ascontiguousarray(
            x[g].T.reshape(KO, P, S).transpose(1, 0, 2)
        ).astype(e4)
        for g in range(B)
    ]
    wo8 = np.ascontiguousarray(
        wo.reshape(NH // 2, P, H).transpose(1, 0, 2)
    ).astype(e4)

    def wslice(w, cols):
        return np.ascontiguousarray(
            w[:, cols].reshape(KO, P, DLOC).transpose(1, 0, 2)
        ).astype(e4)

    import ml_dtypes as _mld
    pat2_host = np.zeros((HD, P), _mld.bfloat16)
    pat2_host[0, :HD] = 1.0
    pat2_host[32, HD:] = 1.0

    in_maps = []
    for c in range(N_CORES):
        g, r = c // RANKS, c % RANKS
        cols = slice(DLOC * r, DLOC * (r + 1))
        in_maps.append(
            {
                "x8": x8g[g],
                "xres": np.ascontiguousarray(x[g, SSH * r : SSH * (r + 1)]),
                "wq": wslice(wq, cols),
                "wk": wslice(wk, cols),
                "wv": wslice(wv, cols),
                "wo": wo8,
                "bq": np.ascontiguousarray(bq[cols]),
                "bk": np.ascontiguousarray(bk[cols]),
                "bv": np.ascontiguousarray(bv[cols]),
                "bo": bo,
                "gamma": gamma,
                "beta": beta,
                "mask": (np.arange(N_CORES) // RANKS == g).astype(np.float32),
                "pat2": pat2_host,
            }
        )
    return in_maps


def run_spmd(inputs, trace=False):
    results = _get_runner().run(make_in_maps(inputs))
    out = np.empty((B, S, H), np.float32)
    for c in range(N_CORES):
        g, r = c // RANKS, c % RANKS
        out[g, SSH * r : SSH * (r + 1)] = results[c]["out"]
    return out, results


def kernel(**inputs) -> np.ndarray:
    out, _ = run_spmd(inputs)
    return out
